# revision 14
# baseline (speedup 1.0000x reference)
import numpy as np

# nn_AttentionLayers_85289460564565
# Full attention layer on 8 NeuronCores, one SPMD Bass kernel, no collectives
# (on-device collective setup through this runtime has a large, unpredictable
# first-run staging cost, so every core gets the full weights + full-batch xT
# and computes K/V redundantly; transfers stay the stable cost instead).
# Sharding: query rows across cores; core c handles batch c//4, rows
# [(c%4)*512, (c%4+1)*512). Each core's xfT ships with its own 512 query rows
# permuted to columns 0..511 so the program is SPMD-uniform; key order follows
# the same permutation and causal masks are driven by shipped index data.
# Talking-heads pre_proj is folded into per-output-head Q scaling (full-dim
# contraction); post_proj is folded into V scaling inside the A@V
# accumulation. Scores are computed transposed [j, i] so softmax sums are
# ones-matmuls; normalization is deferred via a broadcast reciprocal. No PE
# transposes anywhere. A jax.export artifact + the jax persistent compile
# cache under /tmp make warm fresh-process runs skip Bass build and walrus.

B, N, DIM = 2, 2048, 1024
H, DH = 16, 64
MEM = 16
ROT = 32
NCORES = 8
R = 512            # query rows per core
GJ = 4             # cores per batch group (gather width)
JT = 17            # j tiles: 16 x 128 real keys + 1 mem tile (16)
NJ = N + MEM       # 2064
SCALE = DH ** -0.5

_BUILT = {}


def _build_nc():
    import concourse.mybir as mybir
    from concourse import bacc
    from concourse.tile import TileContext

    bf16 = mybir.dt.bfloat16
    f32 = mybir.dt.float32
    Exp = mybir.ActivationFunctionType.Exp
    Ident = mybir.ActivationFunctionType.Identity
    mult = mybir.AluOpType.mult
    add = mybir.AluOpType.add
    is_ge = mybir.AluOpType.is_ge

    nc = bacc.Bacc("TRN2", num_devices=NCORES)

    # ---- I/O ----  (no collectives: weights + full-batch x shipped per core;
    # each core's xfT has its own 512 query rows permuted to columns 0..511,
    # key order per core is the same permutation, masks use shipped indices)
    xfT_d = nc.dram_tensor("xfT", [DIM, N], bf16, kind="ExternalInput")
    wts_d = nc.dram_tensor("wts", [4 * DIM, DIM], bf16, kind="ExternalInput")
    cosP_d = nc.dram_tensor("cosP", [16, N], bf16, kind="ExternalInput")
    sinP_d = nc.dram_tensor("sinP", [16, N], bf16, kind="ExternalInput")
    memKT_d = nc.dram_tensor("memKT", [DIM, MEM], bf16, kind="ExternalInput")
    memV_d = nc.dram_tensor("memV", [MEM, DIM], bf16, kind="ExternalInput")
    precols_d = nc.dram_tensor("precols", [DIM, H], f32, kind="ExternalInput")
    postexp_d = nc.dram_tensor("postexp", [H, DIM], bf16, kind="ExternalInput")
    thresh_d = nc.dram_tensor("thresh", [1, R], f32, kind="ExternalInput")
    jidx_d = nc.dram_tensor("jidx", [128, JT], f32, kind="ExternalInput")
    prot_d = nc.dram_tensor("prot", [128, 128], bf16, kind="ExternalInput")
    bc_d = nc.dram_tensor("bc", [16, 128], bf16, kind="ExternalInput")
    biasrot_d = nc.dram_tensor("biasrot", [128, 1], f32, kind="ExternalInput")
    bot_d = nc.dram_tensor("bot", [128, 8], f32, kind="ExternalInput")
    ones_d = nc.dram_tensor("ones", [128, 1], bf16, kind="ExternalInput")
    yT_d = nc.dram_tensor("yT", [DIM, R], bf16, kind="ExternalOutput")

    with nc.allow_low_precision("bf16 attention kernel"), TileContext(nc) as tc:
        with (
            tc.tile_pool(name="mp", bufs=1) as mp,
            tc.tile_pool(name="ps", bufs=2, space="PSUM") as ps,
        ):
            # tag plan (static SBUF bytes/partition):
            #  w    [128,1024] bf16 x22 = 44K (wq/wk/wv seq -> vt/vpost/postbx -> wo)
            #  hk   [128, 512] bf16 x12 = 12K (raw/t1/t2 -> qpre/rec/recB -> avb/yt)
            #  e    [128, 512] bf16 x18 = 18K (exp tiles per head)
            #  kt   [128,2064] bf16 x8  = 33K
            #  xf   [128,2048] bf16 x8  = 32K (phase 1)
            #  qt   [128, 512] bf16 x8  =  8K
            #  av   [128, 512] f32  x8  = 16K
            #  m01  [128,16,512] bf16   = 16K
            #  cosk/sink [128,2048] bf16 = 8K; misc ~12K   => ~199... trimmed below
            W = dict(tag="w", bufs=22)
            HK = dict(tag="hk", bufs=12)
            E = dict(tag="e", bufs=18)

            ones = mp.tile([128, 1], bf16)
            nc.sync.dma_start(ones[:, :], ones_d[:, :])
            prot = mp.tile([128, 128], bf16)
            nc.sync.dma_start(prot[:, :], prot_d[:, :])
            jidx = mp.tile([128, JT], f32)
            nc.sync.dma_start(jidx[:, :], jidx_d[:, :])
            bot = mp.tile([128, 8], f32)
            nc.sync.dma_start(bot[:, :], bot_d[:, :])
            precols = mp.tile([128, 8, H], f32)
            for c in range(8):
                nc.sync.dma_start(precols[:, c, :], precols_d[c * 128:(c + 1) * 128, :])

            th1 = mp.tile([1, R], f32, **HK)
            nc.sync.dma_start(th1[:, :], thresh_d[:, :])
            threshB = mp.tile([128, R], f32, tag="thb")
            nc.gpsimd.partition_broadcast(threshB[:, :], th1[:, :])

            # causal masks per j-tile (1.0 = visible)
            m01 = mp.tile([128, 16, R], bf16)
            for t in range(16):
                nc.vector.tensor_scalar(
                    out=m01[:, t, :], in0=threshB[:, :],
                    scalar1=jidx[:, t:t + 1], scalar2=None, op0=is_ge)

            # rotary cos/sin expanded to row layout over full N (permuted order)
            bcsel = mp.tile([16, 128], bf16)
            nc.sync.dma_start(bcsel[:, :], bc_d[:, :])
            cP = mp.tile([16, N], bf16, tag="xf", bufs=8)
            nc.sync.dma_start(cP[:, :], cosP_d[:, :])
            sP = mp.tile([16, N], bf16, tag="xf", bufs=8)
            nc.sync.dma_start(sP[:, :], sinP_d[:, :])
            brot = mp.tile([128, 1], f32)
            nc.sync.dma_start(brot[:, :], biasrot_d[:, :])
            cosBk = mp.tile([128, N], bf16, tag="cosk")
            sinBk = mp.tile([128, N], bf16, tag="sink")
            for q in range(4):
                sl = slice(q * R, (q + 1) * R)
                cps = ps.tile([128, R], f32, tag="mm", bufs=4)
                nc.tensor.matmul(cps[:, :], bcsel[:, :], cP[:, sl], start=True, stop=True)
                nc.scalar.activation(cosBk[:, sl], cps[:, :], Ident, bias=brot[:, :])
                sps = ps.tile([128, R], f32, tag="mm", bufs=4)
                nc.tensor.matmul(sps[:, :], bcsel[:, :], sP[:, sl], start=True, stop=True)
                nc.scalar.activation(sinBk[:, sl], sps[:, :], Ident)

            # ================= phase 1: QKV projections + rotary ===========
            qt = [mp.tile([128, R], bf16, tag=f"qt{c}", name=f"qt{c}") for c in range(8)]
            avacc = [mp.tile([128, R], f32, tag=f"av{m}", name=f"av{m}") for m in range(8)]
            for m in range(8):
                nc.gpsimd.memset(avacc[m][:, :], 0.0)

            xf = [mp.tile([128, N], bf16, tag="xf", bufs=8, name=f"xf{c}") for c in range(8)]
            for c in range(8):
                nc.sync.dma_start(xf[c][:, :], xfT_d[c * 128:(c + 1) * 128, :])

            kt = [mp.tile([128, NJ], bf16, tag=f"kt{c}", name=f"kt{c}") for c in range(8)]
            for c in range(8):
                nc.sync.dma_start(kt[c][:, N:NJ], memKT_d[c * 128:(c + 1) * 128, :])
            vt = [mp.tile([128, DIM], bf16, name=f"vt{t}", **W) for t in range(16)]
            vmem = mp.tile([MEM, DIM], bf16)
            nc.sync.dma_start(vmem[:, :], memV_d[:, :])

            def rot_evict(pps, dest_ap, cos_sl, sin_sl):
                raw = mp.tile([128, R], bf16, name="raw", **HK)
                nc.scalar.copy(raw[:, :], pps[:, :])
                shp = ps.tile([128, R], f32, tag="mm", bufs=4)
                nc.tensor.matmul(shp[:, :], prot[:, :], raw[:, :], start=True, stop=True)
                t1 = mp.tile([128, R], bf16, name="t1", **HK)
                nc.vector.tensor_tensor(t1[:, :], raw[:, :], cosBk[:, cos_sl], op=mult)
                t2 = mp.tile([128, R], bf16, name="t2", **HK)
                nc.vector.tensor_tensor(t2[:, :], shp[:, :], sinBk[:, sin_sl], op=mult)
                nc.vector.tensor_tensor(dest_ap, t1[:, :], t2[:, :], op=add)

            # Q: own rows = xf columns [0, R)
            wq = [mp.tile([128, DIM], bf16, name=f"wq{c}", **W) for c in range(8)]
            for c in range(8):
                nc.sync.dma_start(wq[c][:, :], wts_d[c * 128:(c + 1) * 128, :])
            for mt in range(8):
                pps = ps.tile([128, R], f32, tag="mm", bufs=4)
                for c in range(8):
                    nc.tensor.matmul(pps[:, :], wq[c][:, mt * 128:(mt + 1) * 128],
                                     xf[c][:, 0:R], start=(c == 0), stop=(c == 7))
                rot_evict(pps, qt[mt][:, :], slice(0, R), slice(0, R))

            # K: full N columns
            wk = [mp.tile([128, DIM], bf16, name=f"wk{c}", **W) for c in range(8)]
            for c in range(8):
                nc.sync.dma_start(wk[c][:, :], wts_d[DIM + c * 128:DIM + (c + 1) * 128, :])
            for mt in range(8):
                for q in range(4):
                    sl = slice(q * R, (q + 1) * R)
                    pps = ps.tile([128, R], f32, tag="mm", bufs=4)
                    for c in range(8):
                        nc.tensor.matmul(pps[:, :], wk[c][:, mt * 128:(mt + 1) * 128],
                                         xf[c][:, sl], start=(c == 0), stop=(c == 7))
                    rot_evict(pps, kt[mt][:, sl], sl, sl)

            # V: [j, (h,d)] over full N rows
            wv = [mp.tile([128, DIM], bf16, name=f"wv{c}", **W) for c in range(8)]
            for c in range(8):
                nc.sync.dma_start(wv[c][:, :], wts_d[2 * DIM + c * 128:2 * DIM + (c + 1) * 128, :])
            for jm in range(16):
                for nh in range(2):
                    vps = ps.tile([128, R], f32, tag="mm", bufs=4)
                    for c in range(8):
                        nc.tensor.matmul(
                            vps[:, :], xf[c][:, jm * 128:(jm + 1) * 128],
                            wv[c][:, nh * 512:(nh + 1) * 512],
                            start=(c == 0), stop=(c == 7))
                    nc.scalar.copy(vt[jm][:, nh * 512:(nh + 1) * 512], vps[:, :])

            # ================= phase 2: attention per output head ==========
            for k in range(H):
                qpre = [mp.tile([128, R], bf16, name=f"qpre{c}", **HK)
                        for c in range(8)]
                for c in range(8):
                    nc.vector.tensor_scalar(
                        out=qpre[c][:, :], in0=qt[c][:, :],
                        scalar1=precols[:, c, k:k + 1], scalar2=None, op0=mult)
                et = [mp.tile([128, R], bf16, name=f"e{t}", **E)
                      for t in range(JT)]
                sums = ps.tile([1, R], f32, tag="sum", bufs=1)
                for t in range(JT):
                    pt = 128 if t < 16 else MEM
                    scp = ps.tile([128, R], f32, tag="sc", bufs=3)
                    if t < 16:
                        lhs = [kt[c][:, t * 128:(t + 1) * 128] for c in range(8)]
                    else:
                        lhs = [kt[c][:, N:NJ] for c in range(8)]
                    for c in range(8):
                        nc.tensor.matmul(scp[:pt, :], lhs[c], qpre[c][:, :],
                                         start=(c == 0), stop=(c == 7))
                    nc.scalar.activation(et[t][:pt, :], scp[:pt, :], Exp, scale=SCALE)
                    if t < 16:
                        nc.vector.tensor_tensor(
                            et[t][:, :], et[t][:, :], m01[:, t, :], op=mult)
                    nc.tensor.matmul(sums[:, :], ones[:pt, :], et[t][:pt, :],
                                     start=(t == 0), stop=(t == JT - 1))
                rec = mp.tile([1, R], bf16, name="rec", **HK)
                nc.vector.reciprocal(rec[:, :], sums[:, :])
                recB = mp.tile([128, R], bf16, name="recB", **HK)
                nc.gpsimd.partition_broadcast(recB[:, :], rec[:, :])
                for t in range(JT):
                    pt = 128 if t < 16 else MEM
                    nc.vector.tensor_tensor(
                        et[t][:pt, :], et[t][:pt, :], recB[:pt, :], op=mult)

                postrow = mp.tile([1, DIM], bf16, name="postrow", **W)
                nc.sync.dma_start(postrow[:, :], postexp_d[k:k + 1, :])
                postBX = mp.tile([128, DIM], bf16, name="postBX", **W)
                nc.gpsimd.partition_broadcast(postBX[:, :], postrow[:, :])
                for half in range(2):
                    avp = [ps.tile([128, R], f32, tag="mm", bufs=4, name=f"avp{m}")
                           for m in range(4)]
                    for t in range(JT):
                        pt = 128 if t < 16 else MEM
                        vsrc = vt[t] if t < 16 else vmem
                        vp = mp.tile([128, DIM], bf16, name="vp", **W)
                        nc.vector.tensor_tensor(
                            vp[:pt, :], vsrc[:pt, :], postBX[:pt, :], op=mult)
                        for m in range(4):
                            M = half * 4 + m
                            nc.tensor.matmul(
                                avp[m][:, :], vp[:pt, M * 128:(M + 1) * 128],
                                et[t][:pt, :], start=(t == 0), stop=(t == JT - 1))
                    for m in range(4):
                        M = half * 4 + m
                        nc.vector.tensor_tensor(
                            avacc[M][:, :], avacc[M][:, :], avp[m][:, :], op=add)

            # ================= phase 3: output projection ==================
            avb = [mp.tile([128, R], bf16, name=f"avb{m}", **HK) for m in range(8)]
            for m in range(8):
                nc.scalar.copy(avb[m][:, :], avacc[m][:, :])
            wo = [mp.tile([128, DIM], bf16, name=f"wo{c}", **W) for c in range(8)]
            for c in range(8):
                nc.sync.dma_start(
                    wo[c][:, :], wts_d[3 * DIM + c * 128:3 * DIM + (c + 1) * 128, :])
            for M in range(8):
                yps = ps.tile([128, R], f32, tag="mm", bufs=4)
                for c in range(8):
                    nc.tensor.matmul(yps[:, :], wo[c][:, M * 128:(M + 1) * 128],
                                     avb[c][:, :], start=(c == 0), stop=(c == 7))
                yt = mp.tile([128, R], bf16, name="yt", **HK)
                nc.scalar.activation(yt[:, :], yps[:, :], Ident,
                                     bias=bot[:, M:M + 1])
                nc.sync.dma_start(yT_d[M * 128:(M + 1) * 128, :], yt[:, :])

    nc.finalize()
    return nc


def _host_inputs(x, rotary_pos_emb, Wq, Wk, Wv, mem_k, mem_v, pre_proj,
                 post_proj, Wo, bo):
    import ml_dtypes
    bf16 = ml_dtypes.bfloat16
    f32 = np.float32

    x = np.asarray(x, f32)
    WT_all = np.concatenate([
        np.asarray(Wq, f32).T, np.asarray(Wk, f32).T,
        np.asarray(Wv, f32).T, np.asarray(Wo, f32).T], axis=0).astype(bf16)

    ang = np.asarray(rotary_pos_emb, f32).reshape(N, ROT)[:, :16]  # [N, 16]
    cos_all = np.cos(ang).astype(bf16)
    sin_all = np.sin(ang).astype(bf16)

    memKT = np.asarray(mem_k, f32).transpose(0, 2, 1).reshape(DIM, MEM).astype(bf16)
    memV = np.asarray(mem_v, f32).transpose(1, 0, 2).reshape(MEM, DIM).astype(bf16)
    precols = np.repeat(np.asarray(pre_proj, f32), DH, axis=0)  # [DIM, H] f32
    postexp = np.repeat(np.asarray(post_proj, f32), DH, axis=1).astype(bf16)

    prot = np.zeros((128, 128), f32)
    for hh in (0, 64):
        for d in range(16):
            prot[hh + 16 + d, hh + d] = -1.0
            prot[hh + d, hh + 16 + d] = 1.0
    prot = prot.astype(bf16)

    bc = np.zeros((16, 128), f32)
    for hh in (0, 64):
        for j in range(32):
            bc[j % 16, hh + j] = 1.0
    bc = bc.astype(bf16)

    biasrot = np.zeros((128, 1), f32)
    for hh in (0, 64):
        biasrot[hh + 32:hh + 64] = 1.0

    bot = np.asarray(bo, f32).reshape(8, 128).T.copy()  # [128, 8]
    ones = np.ones((128, 1), bf16)

    xbf = x.astype(bf16)
    in_maps = []
    for c in range(NCORES):
        b, r = c // GJ, c % GJ
        own = np.arange(r * R, (r + 1) * R)
        rest = np.concatenate([np.arange(0, r * R), np.arange((r + 1) * R, N)])
        perm = np.concatenate([own, rest])
        jidx = np.empty((128, JT), f32)
        jg = perm.astype(f32)
        for t in range(16):
            jidx[:, t] = jg[t * 128:(t + 1) * 128]
        jidx[:, 16] = -1.0
        in_maps.append({
            "xfT": np.ascontiguousarray(xbf[b][perm, :].T),
            "wts": WT_all,
            "cosP": np.ascontiguousarray(cos_all[perm, :].T),
            "sinP": np.ascontiguousarray(sin_all[perm, :].T),
            "memKT": memKT, "memV": memV,
            "precols": precols, "postexp": postexp,
            "thresh": (r * R + np.arange(R, dtype=f32)).reshape(1, R),
            "jidx": jidx, "prot": prot, "bc": bc, "biasrot": biasrot,
            "bot": bot, "ones": ones,
        })
    return in_maps


IN_ORDER = ["xfT", "wts", "cosP", "sinP", "memKT", "memV", "precols",
            "postexp", "thresh", "jidx", "prot", "bc", "biasrot", "bot",
            "ones"]


def _export_path():
    import hashlib
    import inspect
    try:
        src = inspect.getsource(_build_nc) + inspect.getsource(_host_inputs)
    except Exception:
        src = "nosrc"
    h = hashlib.sha256(src.encode()).hexdigest()[:12]
    return f"/tmp/nn_attn_85289_export_{h}.bin"


def _patch_effect():
    from concourse import bass2jax
    bass2jax.BassEffect.__eq__ = lambda self, other: type(other) is type(self)
    bass2jax.BassEffect.__hash__ = lambda self: hash(type(self))


def _run_fast(concat_in, path):
    import jax
    import jax.numpy as jnp
    from jax.sharding import Mesh, NamedSharding, PartitionSpec

    blob = bytearray(open(path, "rb").read())
    devices = jax.devices()[:NCORES]
    mesh = Mesh(np.asarray(devices), ("core",))
    sh = NamedSharding(mesh, PartitionSpec("core"))
    dev_in = [jax.device_put(a, sh) for a in concat_in]
    zeros = jax.jit(lambda: jnp.zeros((NCORES * DIM, R), jnp.bfloat16),
                    out_shardings=sh)()
    _patch_effect()
    exp = jax.export.deserialize(blob)
    out = exp.call(*dev_in, zeros)
    out = jax.block_until_ready(out)
    return np.asarray(out[0])


def _save_export(nc, concat_in, path):
    import jax
    import ml_dtypes
    from jax.experimental.shard_map import shard_map
    from jax.sharding import Mesh, PartitionSpec

    import concourse.mybir as mybir
    from concourse.bass2jax import _bass_exec_p, install_neuronx_cc_hook, \
        partition_id_tensor

    install_neuronx_cc_hook()
    _patch_effect()
    partition_name = (nc.partition_id_tensor.name
                      if nc.partition_id_tensor else None)
    in_names, out_names, out_avals = [], [], []
    for alloc in nc.m.functions[0].allocations:
        if not isinstance(alloc, mybir.MemoryLocationSet):
            continue
        name = alloc.memorylocations[0].name
        if alloc.kind == "ExternalInput":
            if name != partition_name:
                in_names.append(name)
        elif alloc.kind == "ExternalOutput":
            out_names.append(name)
            out_avals.append(jax.core.ShapedArray(
                tuple(alloc.tensor_shape), mybir.dt.np(alloc.dtype)))
    assert in_names == IN_ORDER and out_names == ["yT"], (in_names, out_names)
    n_params = len(in_names)
    n_outs = len(out_avals)
    in_names = in_names + out_names
    if partition_name:
        in_names.append(partition_name)

    def _body(*args):
        ops = list(args)
        if partition_name:
            ops.append(partition_id_tensor())
        outs = _bass_exec_p.bind(
            *ops, out_avals=tuple(out_avals), in_names=tuple(in_names),
            out_names=tuple(out_names), lowering_input_output_aliases=(),
            sim_require_finite=True, sim_require_nnan=True, nc=nc)
        return tuple(outs)

    devices = jax.devices()[:NCORES]
    mesh = Mesh(np.asarray(devices), ("core",))
    sharded = jax.jit(
        shard_map(_body, mesh=mesh,
                  in_specs=(PartitionSpec("core"),) * (n_params + n_outs),
                  out_specs=(PartitionSpec("core"),) * n_outs,
                  check_rep=False),
        donate_argnums=tuple(range(n_params, n_params + n_outs)),
        keep_unused=True)
    zeros = np.zeros((NCORES * DIM, R), ml_dtypes.bfloat16)
    dsc = jax.export.DisabledSafetyCheck.custom_call("bass_exec")
    exp = jax.export.export(sharded, disabled_checks=[dsc])(*concat_in, zeros)
    tmp = path + ".tmp"
    with open(tmp, "wb") as f:
        f.write(exp.serialize())
    import os
    os.replace(tmp, path)


def kernel(x, rotary_pos_emb, Wq, Wk, Wv, mem_k, mem_v, pre_proj, post_proj,
           Wo, bo):
    import os

    import jax
    try:
        jax.config.update("jax_compilation_cache_dir", "/tmp/jaxcache")
        jax.config.update("jax_persistent_cache_min_entry_size_bytes", 0)
        jax.config.update("jax_persistent_cache_min_compile_time_secs", 0.0)
    except Exception:
        pass

    in_maps = _host_inputs(x, rotary_pos_emb, Wq, Wk, Wv, mem_k, mem_v,
                           pre_proj, post_proj, Wo, bo)
    concat_in = [np.concatenate([np.asarray(in_maps[c][n])
                                 for c in range(NCORES)], axis=0)
                 for n in IN_ORDER]
    path = _export_path()
    yT_all = None
    if os.path.exists(path):
        try:
            yT_all = _run_fast(concat_in, path)
        except Exception:
            yT_all = None
    if yT_all is None:
        from concourse import bass_utils
        if "nc" not in _BUILT:
            _BUILT["nc"] = _build_nc()
        res = bass_utils.run_bass_kernel_spmd(_BUILT["nc"], in_maps,
                                              list(range(NCORES)))
        yT_all = np.concatenate([res.results[c]["yT"]
                                 for c in range(NCORES)], axis=0)
        try:
            _save_export(_BUILT["nc"], concat_in, path)
        except Exception:
            pass

    yT_all = np.asarray(yT_all).reshape(NCORES, DIM, R)
    y = np.empty((B, N, DIM), np.float32)
    for c in range(NCORES):
        b, r = c // GJ, c % GJ
        y[b, r * R:(r + 1) * R, :] = yT_all[c].T.astype(np.float32)
    return y
